# revision 14
# baseline (speedup 1.0000x reference)
"""Trainium2 Bass kernel: causal GQA self-attention
(B=2, T=2048, C=1024, 16 q-heads / 4 kv-heads, rotary + q/k RMS-norm),
sharded over 8 NeuronCores as (batch x kv-group).

Optimized vs baseline:
- Phase 2 restructured to per-128-key-chunk pipeline units (1-bank PSUM
  tiles, bufs=2) so PE scores / ACT exp / PE AV overlap.
- Out-projection for query block j hoisted into the j loop (overlaps with
  attention of block j+1).
- Rope computed as a few large DVE ops over the whole [128,TC,*,64] tensors
  using precomputed [cos|cos] / [sin|-sin] 64-wide tables, instead of ~190
  small per-chunk ops.
"""
import sys
from contextlib import ExitStack

for p in ("/opt/trn_rl_repo", "/root/.axon_site/_ro/trn_rl_repo"):
    if p not in sys.path:
        sys.path.insert(0, p)

import numpy as np
import ml_dtypes

import concourse.bass as bass
import concourse.mybir as mybir
from concourse.tile import TileContext
from concourse.masks import make_identity

F32 = mybir.dt.float32
BF16 = mybir.dt.bfloat16
NPBF16 = ml_dtypes.bfloat16

T, C, HQ, D = 2048, 1024, 4, 64
DQ = HQ * D
TC = T // 128
KC = C // 128
NJ = T // 512
EPS = 1.1920929e-7
NEG = -1e30


def _bcast_ap(sl, n, at=1):
    ap = list(sl.ap)
    ap.insert(at, [0, n])
    return bass.AP(tensor=sl.tensor, offset=sl.offset, ap=ap)


def _split_waits(nc, maxw=1):
    """Walrus in this toolchain allows 1 sem-wait per instruction; split extras
    onto preceding same-engine NoOps."""
    cnt = 0
    for f in nc.m.functions:
        for b in f.blocks:
            il = list(b.instructions)
            out = []
            changed = False
            for inst in il:
                si = inst.sync_info
                waits = list(si.on_wait) if si and si.on_wait else []
                if len(waits) > maxw:
                    chunks = [waits[i:i + maxw] for i in range(0, len(waits), maxw)]
                    for ch in chunks[:-1]:
                        cnt += 1
                        nop = mybir.InstNoOp(name=f"I-waitfix-{cnt}")
                        nop.engine = inst.engine
                        nop.sync_info = mybir.SyncInfo(on_wait=ch, on_update=[])
                        out.append(nop)
                    si.on_wait = chunks[-1]
                    inst.sync_info = si
                    changed = True
                out.append(inst)
            if changed:
                b.instructions = out
    return cnt


def _build_attn(ctx, tc, outs, ins):
    nc = tc.nc
    xT, wq, wkv, wo, cos2, sin2 = (
        ins["xT"], ins["wq"], ins["wkv"], ins["wo"], ins["cos2"], ins["sin2"])
    outT = outs["outT"]

    singles = ctx.enter_context(tc.tile_pool(name="singles", bufs=1))

    ident = singles.tile([128, 128], F32, tag="ident")
    make_identity(nc, ident)
    ones_row = singles.tile([1, 64], F32, tag="ones_row")
    nc.vector.memset(ones_row, 1.0)
    eps_t = singles.tile([128, 1], F32, tag="eps_t")
    nc.vector.memset(eps_t, EPS)

    xsb = singles.tile([128, KC, T], BF16, tag="xsb")
    xTr = xT.rearrange("(a p) t -> p a t", p=128)
    nc.sync.dma_start(out=xsb[:, :, 0:512], in_=xTr[:, :, 0:512])
    wq_sb = singles.tile([128, KC, DQ], BF16, tag="wq_sb")
    nc.sync.dma_start(out=wq_sb, in_=wq.rearrange("(a p) n -> p a n", p=128))
    wkv_sb = singles.tile([128, KC, 128], BF16, tag="wkv_sb")
    nc.sync.dma_start(out=wkv_sb, in_=wkv.rearrange("(a p) n -> p a n", p=128))
    for tq in range(1, 4):
        nc.sync.dma_start(out=xsb[:, :, tq * 512:(tq + 1) * 512],
                          in_=xTr[:, :, tq * 512:(tq + 1) * 512])
    wo_sb = singles.tile([128, 2, C], BF16, tag="wo_sb")
    nc.sync.dma_start(out=wo_sb, in_=wo.rearrange("(a p) o -> p a o", p=128))
    # cos2/sin2 are [2048, 64] tables: [cos|cos] and [sin|-sin]
    cos_sb = singles.tile([128, TC, 64], F32, tag="cos_sb")
    nc.sync.dma_start(out=cos_sb, in_=cos2.rearrange("(a p) d -> p a d", p=128))
    sin_sb = singles.tile([128, TC, 64], F32, tag="sin_sb")
    nc.sync.dma_start(out=sin_sb, in_=sin2.rearrange("(a p) d -> p a d", p=128))

    qt0 = singles.tile([128, T], BF16, tag="qt0")
    qt1 = singles.tile([128, T], BF16, tag="qt1")
    kt2 = singles.tile([128, T], BF16, tag="kt2")
    v_sb = singles.tile([128, TC, 65], BF16, tag="v_sb")
    nc.vector.memset(v_sb[:, :, 64:65], 1.0)
    yt0 = singles.tile([128, T], BF16, tag="yt0")
    yt1 = singles.tile([128, T], BF16, tag="yt1")
    qts = (qt0, qt1)
    yts = (yt0, yt1)

    # ---- Phase 1: projections + rope + rms + transposes, in two halves
    # so attention on the first half overlaps projection of the second ----
    with (
        tc.tile_pool(name="scratch", bufs=1) as scratch,
        tc.tile_pool(name="pp", bufs=3, space="PSUM") as pp,
        tc.tile_pool(name="ropep", bufs=2) as ropep,
        tc.tile_pool(name="sqpool", bufs=2) as sqpool,
        tc.tile_pool(name="tpp", bufs=2, space="PSUM") as tpp,
    ):
        q2 = scratch.tile([128, TC, DQ], F32, tag="q2")
        kn = scratch.tile([128, TC, 128], F32, tag="kn")
        mv = scratch.tile([128, TC, HQ], F32, tag="mv")
        sd = scratch.tile([128, TC, HQ], F32, tag="sd")
        rsq = scratch.tile([128, TC, HQ], F32, tag="rsq")
        mvk = scratch.tile([128, TC, 1], F32, tag="mvk")
        sdk = scratch.tile([128, TC, 1], F32, tag="sdk")
        rsk = scratch.tile([128, TC, 1], F32, tag="rsk")
        q4 = q2.rearrange("p t (h d) -> p t h d", h=HQ)
        k4 = kn.rearrange("p t (c d) -> p t c d", d=64)

        for qi, (ts, te) in enumerate(
                ((0, 4), (4, 8), (8, 12), (12, 16))):
            th = te - ts
            for t_ in range(ts, te):
                qps = pp.tile([128, DQ], F32, tag="qps")
                for kc in range(KC):
                    nc.tensor.matmul(
                        qps, xsb[:, kc, t_ * 128:(t_ + 1) * 128], wq_sb[:, kc, :],
                        start=(kc == 0), stop=(kc == KC - 1))
                nc.scalar.copy(q2[:, t_, :], qps)

                kvps = pp.tile([128, 128], F32, tag="kvps")
                for kc in range(KC):
                    nc.tensor.matmul(
                        kvps, xsb[:, kc, t_ * 128:(t_ + 1) * 128], wkv_sb[:, kc, :],
                        start=(kc == 0), stop=(kc == KC - 1))
                nc.scalar.copy(v_sb[:, t_, 0:64], kvps[:, 64:128])
                nc.scalar.copy(kn[:, t_, 0:64], kvps[:, 0:64])

            # rope on this half with [cos|cos], [sin|-sin] tables
            q4h = q4[:, ts:te]
            tmp = ropep.tile([128, th, HQ, 64], F32, tag="tmp")
            nc.vector.tensor_mul(tmp[:, :, :, 0:32], q4h[:, :, :, 32:64],
                                 _bcast_ap(sin_sb[:, ts:te, 0:32], HQ, at=2))
            nc.vector.tensor_mul(tmp[:, :, :, 32:64], q4h[:, :, :, 0:32],
                                 _bcast_ap(sin_sb[:, ts:te, 32:64], HQ, at=2))
            nc.vector.tensor_mul(q4h, q4h, _bcast_ap(cos_sb[:, ts:te], HQ, at=2))
            nc.vector.tensor_add(q4h, q4h, tmp)

            ktmp = tmp[:, :, 0, :]
            nc.gpsimd.tensor_mul(ktmp[:, :, 0:32], k4[:, ts:te, 0, 32:64],
                                 sin_sb[:, ts:te, 0:32])
            nc.gpsimd.tensor_mul(ktmp[:, :, 32:64], k4[:, ts:te, 0, 0:32],
                                 sin_sb[:, ts:te, 32:64])
            nc.gpsimd.tensor_mul(k4[:, ts:te, 0, :], k4[:, ts:te, 0, :],
                                 cos_sb[:, ts:te])
            nc.gpsimd.tensor_add(k4[:, ts:te, 0, :], k4[:, ts:te, 0, :], ktmp)

            # rms on this half
            sq_ = sqpool.tile([128, th, DQ], F32, tag="sq")
            nc.vector.tensor_mul(sq_, q2[:, ts:te, :], q2[:, ts:te, :])
            nc.vector.tensor_reduce(
                mv[:, ts:te], sq_.rearrange("p t (h d) -> p t h d", d=D),
                axis=mybir.AxisListType.X, op=mybir.AluOpType.add)
            nc.scalar.activation(sd[:, ts:te], mv[:, ts:te],
                                 mybir.ActivationFunctionType.Sqrt,
                                 bias=eps_t, scale=1.0 / D)
            nc.vector.reciprocal(rsq[:, ts:te], sd[:, ts:te])
            nc.vector.tensor_mul(
                q4h, q4h, _bcast_ap(rsq[:, ts:te], D, at=3))
            sqk = sqpool.tile([128, th, 64], F32, tag="sqk")
            nc.gpsimd.tensor_mul(sqk, k4[:, ts:te, 0, :], k4[:, ts:te, 0, :])
            nc.vector.tensor_reduce(mvk[:, ts:te], sqk,
                                    axis=mybir.AxisListType.X,
                                    op=mybir.AluOpType.add)
            nc.scalar.activation(sdk[:, ts:te], mvk[:, ts:te],
                                 mybir.ActivationFunctionType.Sqrt,
                                 bias=eps_t, scale=1.0 / D)
            nc.vector.reciprocal(rsk[:, ts:te], sdk[:, ts:te])
            rskh = rsk[:, ts:te, :]
            rb = bass.AP(tensor=rskh.tensor, offset=rskh.offset,
                         ap=[rskh.ap[0], rskh.ap[1], [0, 64]])
            nc.gpsimd.tensor_mul(k4[:, ts:te, 0, :], k4[:, ts:te, 0, :], rb)
            nc.gpsimd.tensor_copy(k4[:, ts:te, 1, :], k4[:, ts:te, 0, :])

            # transposes for this half: pair two t-chunks per PSUM tile so
            # each SBUF copy moves 256 columns
            for t2 in range(ts, te, 2):
                for fs in range(2):
                    tps = tpp.tile([128, 2, 128], F32, tag="tps")
                    for i in range(2):
                        nc.tensor.transpose(
                            tps[:, i, :], q2[:, t2 + i, fs * 128:(fs + 1) * 128],
                            ident)
                    cp = nc.scalar.copy if (t2 % 4) else nc.vector.tensor_copy
                    cp(qts[fs][:, t2 * 128:(t2 + 2) * 128], tps)
                tps2 = tpp.tile([128, 2, 128], F32, tag="tps")
                for i in range(2):
                    nc.tensor.transpose(tps2[:, i, :], kn[:, t2 + i, :], ident)
                cp = nc.vector.tensor_copy if (t2 % 4) else nc.scalar.copy
                cp(kt2[:, t2 * 128:(t2 + 2) * 128], tps2)

    # ---- Phase 2+3: attention with fused out-projection per query block ----
    with (
        tc.tile_pool(name="ptp", bufs=6) as ptp,
        tc.tile_pool(name="smallp", bufs=4) as smallp,
        tc.tile_pool(name="s1p", bufs=4, space="PSUM") as s1p,
        tc.tile_pool(name="o65p", bufs=2, space="PSUM") as o65p,
        tc.tile_pool(name="opp", bufs=2, space="PSUM") as opp,
        tc.tile_pool(name="osp", bufs=3) as osp,
    ):
        for j in range(NJ):
            for h in range(HQ):
                pair, base = h // 2, (h % 2) * 64
                tp = (base, 0) if base else None
                o65 = o65p.tile([65, 512], F32, tag="o65")
                nch = 4 * (j + 1)
                for c in range(nch):
                    # diagonal chunk i only touches queries f >= 128*i: slice
                    # scores/exp/AV to the live column range
                    qoff = max(0, (c - 4 * j)) * 128 if c >= 4 * j else 0
                    qsl = slice(j * 512 + qoff, (j + 1) * 512)
                    s1 = s1p.tile([128, 512], F32, tag="s1")
                    nc.tensor.matmul(
                        s1[:, qoff:512],
                        kt2[base:base + 64, c * 128:(c + 1) * 128],
                        qts[pair][base:base + 64, qsl],
                        start=True, stop=True, tile_position=tp)
                    pt = ptp.tile([128, 512], BF16, tag="pt")
                    nc.scalar.activation(pt[:, qoff:512], s1[:, qoff:512],
                                         mybir.ActivationFunctionType.Exp,
                                         scale=0.125)
                    if c >= 4 * j:
                        # causal: zero keys above the diagonal (post-exp);
                        # sliced AP starts at f'=0 == f-qoff, and
                        # qoff == 128*(c-4j), so the compare base is 0
                        nc.gpsimd.affine_select(
                            out=pt[:, qoff:512], in_=pt[:, qoff:512],
                            compare_op=mybir.AluOpType.is_ge,
                            fill=0.0, base=0,
                            pattern=[[1, 512 - qoff]], channel_multiplier=-1)
                    nc.tensor.matmul(
                        o65[:, qoff:512], v_sb[:, c, 0:65], pt[:, qoff:512],
                        start=(c == 0), stop=(c == nch - 1))
                rec = smallp.tile([1, 512], F32, tag="rec")
                nc.vector.reciprocal(rec, o65[64:65, :])
                bct = opp.tile([128, 512], F32, tag="ops")
                bc = bct[0:64, :]
                nc.tensor.matmul(bc, ones_row, rec, start=True, stop=True)
                bcs = smallp.tile([64, 512], F32, tag="bcs")
                nc.vector.tensor_copy(bcs, bc)
                nc.vector.tensor_mul(
                    yts[pair][base:base + 64, j * 512:(j + 1) * 512],
                    o65[0:64, :], bcs)
            # out-projection for query block j
            for m in range(8):
                ops_ = opp.tile([128, 512], F32, tag="ops")
                for fc in range(2):
                    nc.tensor.matmul(
                        ops_, wo_sb[:, fc, m * 128:(m + 1) * 128],
                        yts[fc][:, j * 512:(j + 1) * 512],
                        start=(fc == 0), stop=(fc == 1))
                ot = osp.tile([128, 512], F32, tag="ot")
                nc.vector.tensor_copy(ot, ops_)
                nc.sync.dma_start(
                    out=outT[m * 128:(m + 1) * 128, j * 512:(j + 1) * 512],
                    in_=ot)


def _build_nc(loop_n=0, split=True):
    """loop_n=0: single-shot kernel (grading path). loop_n=N>0: body wrapped
    in a hardware For-loop executing N times — used by test.py to measure
    per-iteration device time with launch overhead amortized out."""
    nc = bass.Bass("TRN2", target_bir_lowering=False, debug=False, num_devices=8)
    ins = {
        "xT": nc.dram_tensor("xT", [1024, 2048], BF16, kind="ExternalInput").ap(),
        "wq": nc.dram_tensor("wq", [1024, 256], BF16, kind="ExternalInput").ap(),
        "wkv": nc.dram_tensor("wkv", [1024, 128], BF16, kind="ExternalInput").ap(),
        "wo": nc.dram_tensor("wo", [256, 1024], BF16, kind="ExternalInput").ap(),
        "cos2": nc.dram_tensor("cos2", [2048, 64], F32, kind="ExternalInput").ap(),
        "sin2": nc.dram_tensor("sin2", [2048, 64], F32, kind="ExternalInput").ap(),
    }
    outs = {"outT": nc.dram_tensor("outT", [1024, 2048], F32,
                                   kind="ExternalOutput").ap()}
    with TileContext(nc) as tc:
        if loop_n:
            # PE/ACT bodies exceed one 16KiB IRAM block; hint the back-edge
            # so the timing loop doesn't pay a ~4us I$ miss per iteration
            with tc.For_i(0, loop_n, 1,
                          hint_engines=(mybir.EngineType.PE,
                                        mybir.EngineType.Activation)):
                with ExitStack() as ctx:
                    _build_attn(ctx, tc, outs, ins)
        else:
            with ExitStack() as ctx:
                _build_attn(ctx, tc, outs, ins)
    if split:
        _split_waits(nc, maxw=1)
    return nc


def _shard_inputs(inputs, b, g):
    x, cos, sin = inputs["x"], inputs["cos"], inputs["sin"]
    Wq, Wk, Wv, Wo = inputs["Wq"], inputs["Wk"], inputs["Wv"], inputs["Wo"]
    qs, ks = slice(g * 256, (g + 1) * 256), slice(g * 64, (g + 1) * 64)
    c1 = np.asarray(cos[0, :, 0, :], dtype=np.float32)
    s1 = np.asarray(sin[0, :, 0, :], dtype=np.float32)
    return {
        "xT": np.ascontiguousarray(np.asarray(x[b]).T.astype(NPBF16)),
        "wq": np.ascontiguousarray(np.asarray(Wq[qs]).T.astype(NPBF16)),
        "wkv": np.ascontiguousarray(np.concatenate(
            [np.asarray(Wk[ks]).T, np.asarray(Wv[ks]).T], axis=1).astype(NPBF16)),
        "wo": np.ascontiguousarray(np.asarray(Wo[:, qs]).T.astype(NPBF16)),
        "cos2": np.ascontiguousarray(np.concatenate([c1, c1], axis=1)),
        "sin2": np.ascontiguousarray(np.concatenate([s1, -s1], axis=1)),
    }


_STATE = {}


def _get_state(loop_n=0):
    if loop_n in _STATE:
        return _STATE[loop_n]
    import jax
    from jax.sharding import Mesh, PartitionSpec, NamedSharding
    from jax.experimental.shard_map import shard_map
    from concourse.bass2jax import (
        _bass_exec_p, install_neuronx_cc_hook, partition_id_tensor)

    install_neuronx_cc_hook()
    nc = _build_nc(loop_n)
    pname = nc.partition_id_tensor.name if nc.partition_id_tensor else None

    in_names, out_names, out_avals, zero_outs = [], [], [], []
    for alloc in nc.m.functions[0].allocations:
        if not isinstance(alloc, mybir.MemoryLocationSet):
            continue
        name = alloc.memorylocations[0].name
        if alloc.kind == "ExternalInput":
            if name != pname:
                in_names.append(name)
        elif alloc.kind == "ExternalOutput":
            out_names.append(name)
            shape = tuple(alloc.tensor_shape)
            dtype = mybir.dt.np(alloc.dtype)
            out_avals.append(jax.core.ShapedArray(shape, dtype))
            zero_outs.append(np.zeros(shape, dtype))
    n_params = len(in_names)
    all_names = in_names + out_names
    if pname is not None:
        all_names = all_names + [pname]

    def _body(*args):
        operands = list(args)
        if pname is not None:
            operands.append(partition_id_tensor())
        outs = _bass_exec_p.bind(
            *operands, out_avals=tuple(out_avals), in_names=tuple(all_names),
            out_names=tuple(out_names), lowering_input_output_aliases=(),
            sim_require_finite=True, sim_require_nnan=True, nc=nc)
        return tuple(outs)

    devices = jax.devices()[:8]
    mesh = Mesh(np.asarray(devices), ("core",))
    specs = (PartitionSpec("core"),) * (n_params + 1)
    sharded = jax.jit(shard_map(_body, mesh=mesh, in_specs=specs,
                                out_specs=(PartitionSpec("core"),),
                                check_rep=False))
    sharding = NamedSharding(mesh, PartitionSpec("core"))
    zeros = jax.device_put(
        np.zeros((8 * 1024, 2048), np.float32), sharding)
    _STATE[loop_n] = dict(sharded=sharded, sharding=sharding,
                          in_names=in_names, zeros=zeros, jax=jax)
    return _STATE[loop_n]


def _run_device(in_maps, loop_n=0):
    st = _get_state(loop_n)
    jax = st["jax"]
    concat_in = [np.concatenate([m[n] for m in in_maps], axis=0)
                 for n in st["in_names"]]
    dev_in = [jax.device_put(a, st["sharding"]) for a in concat_in]
    out = st["sharded"](*dev_in, st["zeros"])[0]
    return np.asarray(out).reshape(8, 1024, 2048)


def kernel(**inputs) -> np.ndarray:
    inputs = {k: np.asarray(v) for k, v in inputs.items()}
    in_maps = [_shard_inputs(inputs, b, g) for b in range(2) for g in range(4)]
    arr = _run_device(in_maps)
    out = np.zeros((2, 2048, 1024), np.float32)
    for c in range(8):
        out[c // 4] += arr[c].T
    return out
